# revision 80
# baseline (speedup 1.0000x reference)
"""Multi-head self-attention (B=4, S=2048, D=1024, H=16, causal) on 8 TRN2 NeuronCores.

Sharding: tensor-parallel over heads (2 heads/core) for QKV projection + attention.
Per-batch AllToAll redistributes attention outputs so the output projection is
token-parallel (each core owns a 256-token slice of every batch). No reduction
collective needed.

Orientation: everything is computed transposed (feature-major) so all matmuls
contract over the partition dimension with 512-wide free dims:
  Q^T/K^T/V^T [hd, tok] = W^T x^T  (x^T supplied by host, bf16; V^T then
                                    PE-transposed per chunk to key-major
                                    V_aug tiles with a fused ones column)
  S^T [k, q]  = K^T-block as lhsT, Q^T as rhs (keys on partitions)
  P^T = exp(S^T/8) on ScalarE -> bf16, causal-masked by a DVE multiply with
        4 precomputed 0/1 diagonal-mask tiles (all-bf16 all-SBUF -> 2x mode)
  attn^T [hd, q] += V_aug^T P^T   (fused ones-column in V gives denominators)
  out [tok, d] = (attn^T chunks as lhsT) @ W_out

All matmul operands are bf16 (fp32 PSUM accumulation): the HAM power manager
clocks the PE at ~1.95 GHz for bf16 streams vs 1.2 GHz for fp32/fp32r.

Pipeline: batch b's attention interleaves batch b+1's QKV projection as PE
filler; a2a(b) fires right after batch b's attention; out-proj for batches
0-2 interleaves into batches 1 and 3 as more filler; only a2a(3)+outproj(3)
remain in the tail.
"""

import numpy as np

B, S, D, H = 4, 2048, 1024, 16
HD = D // H            # 64
CORES = 8
P = 128
TOK = B * S            # 8192 tokens (flattened b,s)
TPC = TOK // CORES     # 1024 tokens per core for out-proj
SPC = S // CORES       # 256-token slice of each batch owned per core
HPC = H // CORES       # 2 heads per core
QC = 512               # query chunk
NQC = S // QC          # 4 q-chunks per sequence
KB = S // P            # 16 key blocks per sequence
DCH = D // P           # 8 contraction chunks over D
DIAG = QC // P         # 4 diagonal key-blocks per q-chunk

_CACHE = {}


def _build():
    import concourse.mybir as mybir
    import concourse.tile as tile
    from concourse import bacc

    F32 = mybir.dt.float32
    BF16 = mybir.dt.bfloat16
    EXP = mybir.ActivationFunctionType.Exp
    MULT = mybir.AluOpType.mult

    nc = bacc.Bacc("TRN2", target_bir_lowering=False, debug=False, num_devices=CORES)

    xt = nc.dram_tensor("xt", [D, TOK], BF16, kind="ExternalInput").ap()
    wqkv = nc.dram_tensor("wqkv", [D, 3 * P], BF16, kind="ExternalInput").ap()
    bqkv = nc.dram_tensor("bqkv", [3 * P], F32, kind="ExternalInput").ap()
    identf = nc.dram_tensor("identf", [P, HD], F32, kind="ExternalInput").ap()
    wout = nc.dram_tensor("wout", [D, D], BF16, kind="ExternalInput").ap()
    bout = nc.dram_tensor("bout", [D], BF16, kind="ExternalInput").ap()
    onesd = nc.dram_tensor("onesd", [P, P], BF16, kind="ExternalInput").ap()
    maskd = nc.dram_tensor("maskd", [P, DIAG, QC], BF16, kind="ExternalInput").ap()
    out = nc.dram_tensor("out", [TPC, D], BF16, kind="ExternalOutput").ap()

    # A2A buffers: 4 exchanges (batches {0,1} / {2} / {3: chunks 3,1} /
    # {3: chunks 2,0}) so only the small final one is tail-exposed.
    # Slot j = this core's 2 heads x core j's token slices. For batch 3 the
    # ownership is 64-token interleaved so each exchange covers 2 chunks.
    cc_in = [
        nc.dram_tensor("cc_in01", [CORES, P, 2 * SPC], BF16),
        nc.dram_tensor("cc_in2", [CORES, P, SPC], BF16),
        nc.dram_tensor("cc_in3a", [CORES, P, P], BF16),
        nc.dram_tensor("cc_in3b", [CORES, P, P], BF16),
    ]
    cc_out = [
        nc.dram_tensor("cc_out01", [CORES, P, 2 * SPC], BF16),
        nc.dram_tensor("cc_out2", [CORES, P, SPC], BF16),
        nc.dram_tensor("cc_out3a", [CORES, P, P], BF16),
        nc.dram_tensor("cc_out3b", [CORES, P, P], BF16),
    ]

    with tile.TileContext(nc) as tc:
        with (
            tc.tile_pool(name="const", bufs=1) as const,
            tc.tile_pool(name="xpool", bufs=8) as xpool,
            tc.tile_pool(name="slab", bufs=2) as slab,
            tc.tile_pool(name="vpool", bufs=2) as vpool,
            tc.tile_pool(name="apool", bufs=2) as apool,
            tc.tile_pool(name="ppool", bufs=8) as ppool,
            tc.tile_pool(name="epi", bufs=3) as epi,
            tc.tile_pool(name="ps_st", bufs=3, space="PSUM") as ps_st,
            tc.tile_pool(name="ps_ot", bufs=2, space="PSUM") as ps_ot,
        ):
            # bias first (gates the opening bias-add), then W_qkv with the Q
            # columns leading; the masks/identity/ones aren't needed until
            # ~10us in, so they load after the startup-critical weights
            bq_t = const.tile([P, 3], F32)
            nc.sync.dma_start(bq_t[:], bqkv.rearrange("(s p) -> p s", p=P))
            wq_t = const.tile([P, DCH, 3 * P], BF16)
            wq_src = wqkv.rearrange("(o p) c -> p o c", p=P)
            for s_i in range(3):  # Q, then K, then V columns
                nc.sync.dma_start(
                    wq_t[:, :, s_i * P : (s_i + 1) * P],
                    wq_src[:, :, s_i * P : (s_i + 1) * P],
                )
            ones_t = const.tile([P, P], BF16)
            nc.sync.dma_start(ones_t[:], onesd[:])
            idf_t = const.tile([P, HD], F32)
            nc.sync.dma_start(idf_t[:], identf[:])
            mask_t = const.tile([P, DIAG, QC], BF16)
            nc.sync.dma_start(mask_t[:], maskd[:])
            slabs = {}

            def make_slabs(b):
                vts = []
                for h in range(HPC):
                    vt = vpool.tile([P, KB, 66], BF16, tag=f"v2_{h}", name=f"v2_{h}_{b}")
                    nc.vector.tensor_copy(vt[:, :, 64:65], ones_t[:, 0:KB, None])
                    vts.append(vt)
                return (
                    slab.tile([P, S], BF16, tag="q2t", name=f"q2t{b}"),
                    slab.tile([P, S], BF16, tag="k2t", name=f"k2t{b}"),
                    slab.tile([P, S], F32, tag="v2t", name=f"v2t{b}"),
                    vts,
                )

            def xt_load(b, tc_i):
                """Prefetch the x^T chunk for (b, tc_i): issued a whole batch
                ahead so PE never waits on it even when a collective is
                hogging DMA bandwidth."""
                xt_t = xpool.tile([P, DCH, QC], BF16, tag="xt", name=f"xt_{b}_{tc_i}")
                t0 = b * S
                src = xt[:, t0 + tc_i * QC : t0 + (tc_i + 1) * QC].rearrange(
                    "(o p) t -> p o t", p=P
                )
                nc.sync.dma_start(xt_t[:, 0 : DCH // 2, :], src[:, 0 : DCH // 2, :])
                nc.sync.dma_start(xt_t[:, DCH // 2 :, :], src[:, DCH // 2 :, :])
                return xt_t

            def qkv_pieces(b, tc_i, xt_t):
                """QKV projection for token chunk tc_i of batch b, as 3 slab-level
                filler pieces operating on the prefetched x^T chunk."""
                q2t, k2t, v2t, vts = slabs[b]

                def piece(s_i, dst):
                    psum3 = ps_st.tile([P, 2, QC], F32, tag="st", name="qkvps")
                    psum = psum3[:, 0, :]
                    for dc in range(DCH):
                        nc.tensor.matmul(
                            psum[:],
                            wq_t[:, dc, s_i * P : (s_i + 1) * P],
                            xt_t[:, dc],
                            start=(dc == 0),
                            stop=(dc == DCH - 1),
                        )
                    nc.vector.tensor_scalar_add(
                        dst[:, tc_i * QC : (tc_i + 1) * QC],
                        psum[:],
                        bq_t[:, s_i : s_i + 1],
                    )
                    if s_i == 2:
                        # transpose this chunk's V^T block to key-major V_aug
                        # tiles (fp32 PE transpose; spread per-chunk so it
                        # rides the filler stream instead of batch bursts)
                        for h in range(HPC):
                            pst = ps_st.tile([P, 2, QC], F32, tag="st", name="vtp")
                            for j in range(DIAG):
                                kb = tc_i * DIAG + j
                                nc.tensor.transpose(
                                    pst[:, 0, j * HD : (j + 1) * HD],
                                    v2t[
                                        h * HD : (h + 1) * HD,
                                        kb * P : (kb + 1) * P,
                                    ],
                                    idf_t[h * HD : (h + 1) * HD, :],
                                )
                            nc.vector.tensor_copy(
                                vts[h][:, tc_i * DIAG : (tc_i + 1) * DIAG, 0:HD],
                                pst[:, 0, 0 : DIAG * HD].rearrange(
                                    "p (a b) -> p a b", b=HD
                                ),
                            )

                return [
                    (lambda s_i=s_i, dst=dst: piece(s_i, dst))
                    for s_i, dst in enumerate((q2t, k2t, v2t))
                ]

            def attention_qc(b, qc_i, filler=None):
                """Attention for q-chunk qc_i of batch b, both heads fused.

                filler: deque of independent-PE-work thunks; one is popped
                every 2nd group to plug exp-wait gaps (keeps the PE stream
                dense so the HAM clock stays up).
                """
                q2t, k2t, _, vts = slabs[b]
                qsl = slice(qc_i * QC, (qc_i + 1) * QC)
                nkb = (qc_i + 1) * DIAG
                otp = [
                    ps_ot.tile([P, QC], F32, tag="ot", name=f"ot{h}")
                    for h in range(HPC)
                ]
                for kb2 in range(nkb // 2):
                    tiles = []
                    for h in range(HPC):
                        stp = ps_st.tile([P, 2, QC], F32, tag="st", name=f"st{h}")
                        pt = ppool.tile([P, 2, QC], BF16, tag="pt", name=f"pt{h}")
                        tiles.append((stp, pt))
                    # scores: (headA, headB) pairs run concurrently (row groups 0/64)
                    for j in range(2):
                        kb = kb2 * 2 + j
                        for h in range(HPC):
                            hof = h * HD
                            nc.tensor.matmul(
                                tiles[h][0][:, j, :],
                                k2t[hof : hof + HD, kb * P : (kb + 1) * P],
                                q2t[hof : hof + HD, qsl],
                                start=True,
                                stop=True,
                                tile_position=(hof, 0),
                            )
                    for h in range(HPC):
                        stp, pt = tiles[h]
                        d0 = kb2 * 2 - qc_i * DIAG
                        if d0 + 1 >= 0:  # group touches the causal diagonal
                            # exp only the rectangle that can survive the
                            # mask; gpsimd (idle) zero-fills the fully-masked
                            # columns so the mask multiply sees no garbage.
                            # Shortens the chunk-end exp on the critical path.
                            for j in range(2):
                                z = P * (d0 + j)
                                if z > 0:
                                    nc.gpsimd.memset(pt[:, j, 0:z], 0.0)
                                nc.scalar.activation(
                                    pt[:, j, z:], stp[:, j, z:], EXP, scale=0.125
                                )
                            nc.vector.tensor_tensor(
                                pt[:], pt[:], mask_t[:, d0 : d0 + 2, :], MULT
                            )
                        else:
                            nc.scalar.activation(pt[:], stp[:], EXP, scale=0.125)
                    # bridge the exp->attnV latency with filler PE work; the
                    # in-order PE would otherwise stall on the last group of
                    # each chunk (nothing queued behind the diagonal's mask)
                    last = kb2 == nkb // 2 - 1
                    if filler and (last or kb2 % 2 == 0 or len(filler) > 8):
                        filler.popleft()()
                    for h in range(HPC):
                        pt = tiles[h][1]
                        for j in range(2):
                            kb = kb2 * 2 + j
                            nc.tensor.matmul(
                                otp[h][0:65, :],
                                vts[h][:, kb, 0:65],
                                pt[:, j, :],
                                start=(kb == 0),
                                stop=(kb == nkb - 1),
                            )
                for h in range(HPC):
                    hof = h * HD
                    # normalize by denominators (row 64): bcast via K=1 matmul
                    den_r = epi.tile([P, QC], BF16, tag="den_r", name="den_r")
                    nc.vector.tensor_copy(den_r[64:65, :], otp[h][64:65, :])
                    dbc3 = ps_st.tile([P, 2, QC], F32, tag="st", name="dbc3")
                    dbc = dbc3[:, 0, :]
                    nc.tensor.matmul(
                        dbc[0:HD, :], ones_t[64:65, 0:HD], den_r[64:65, :],
                        start=True, stop=True,
                    )
                    rden_s = epi.tile([HD, QC], F32, tag="rden_s", name="rden_s")
                    nc.vector.reciprocal_approx_fast(rden_s[:], dbc[0:HD, :])
                    attn_s = epi.tile([HD, QC], BF16, tag="attn_s", name="attn_s")
                    nc.vector.tensor_tensor(attn_s[:], otp[h][0:HD, :], rden_s[:], MULT)
                    # scatter token slices to the A2A input. Exchange 0
                    # carries batches 0+1 side by side; batch 3 is 64-token
                    # interleaved across two half-exchanges (3a: chunks 3,1;
                    # 3b: chunks 2,0) so 3a can fly mid-batch.
                    if b < 3:
                        cci = cc_in[0] if b < 2 else cc_in[1]
                        cof = (b % 2) * SPC if b < 2 else 0
                        for sl in range(2):
                            nc.sync.dma_start(
                                cci[
                                    2 * qc_i + sl,
                                    hof : hof + HD,
                                    cof : cof + SPC,
                                ],
                                attn_s[:, sl * SPC : (sl + 1) * SPC],
                            )
                    else:
                        cci = cc_in[2] if qc_i % 2 else cc_in[3]
                        cof = 0 if qc_i >= 2 else HD
                        for j in range(CORES):
                            nc.sync.dma_start(
                                cci[j, hof : hof + HD, cof : cof + HD],
                                attn_s[:, j * HD : (j + 1) * HD],
                            )

            def a2a(e):
                nc.gpsimd.collective_compute(
                    "AllToAll",
                    mybir.AluOpType.bypass,
                    replica_groups=[list(range(CORES))],
                    ins=[cc_in[e].ap().opt()],
                    outs=[cc_out[e].ap().opt()],
                )

            def outproj_pieces(b):
                """Output projection for this core's 256-token slice of batch b,
                as filler pieces. First piece loads the exchanged activations;
                the rest each compute one [128 tok, 512 feat] psum group."""
                cco = cc_out[0] if b < 2 else cc_out[b - 1]
                cof = (b % 2) * SPC if b < 2 else 0
                state = {}

                def load_piece():
                    at_b = []
                    for j in range(CORES):
                        a_t = apool.tile([P, SPC], BF16, tag=f"at{j}", name=f"at{b}_{j}")
                        nc.sync.dma_start(a_t[:], cco[j, :, cof : cof + SPC])
                        at_b.append(a_t)
                    state["at"] = at_b

                def group_piece(tb, nb_i):
                    at_b = state["at"]
                    psum3 = ps_st.tile([P, 2, QC], F32, tag="st", name="opsum")
                    psum = psum3[:, 0, :]
                    for j in range(CORES):
                        nc.tensor.matmul(
                            psum[:],
                            at_b[j][:, tb * P : (tb + 1) * P],
                            wo_t[:, j, nb_i * QC : (nb_i + 1) * QC],
                            start=(j == 0),
                            stop=False,
                        )
                    nc.tensor.matmul(
                        psum[:],
                        ones_t[0:1, 0:P],
                        bo_t[0:1, nb_i * QC : (nb_i + 1) * QC],
                        start=False,
                        stop=True,
                    )
                    o_s = epi.tile([P, QC], BF16, tag="o_s", name="o_s")
                    nc.vector.tensor_copy(o_s[:], psum[:])
                    nc.sync.dma_start(
                        out[
                            b * SPC + tb * P : b * SPC + (tb + 1) * P,
                            nb_i * QC : (nb_i + 1) * QC,
                        ],
                        o_s[:],
                    )

                return [load_piece] + [
                    (lambda tb=tb, nb_i=nb_i: group_piece(tb, nb_i))
                    for tb in range(SPC // P)
                    for nb_i in range(D // QC)
                ]

            # software pipeline: qkv(0) fully, then per batch interleave qkv(b+1)
            slabs[0] = make_slabs(0)
            xts0 = [xt_load(0, tc_i) for tc_i in range(NQC)]
            ps0 = [qkv_pieces(0, tc_i, xts0[tc_i]) for tc_i in range(NQC)]
            # s-major: all Q pieces first (they only need the Q weight
            # columns, already resident) — covers the K/V weight loads
            for s_i in range(3):
                for tc_i in range(NQC):
                    ps0[tc_i][s_i]()
            # W_out loads off the startup critical path (needed ~batch 2)
            wo_t = const.tile([P, DCH, D], BF16)
            nc.sync.dma_start(wo_t[:], wout.rearrange("(o p) d -> p o d", p=P))
            bo_t = const.tile([1, D], BF16)
            nc.sync.dma_start(bo_t[:], bout[None, :])

            from collections import deque

            QC_ORDER = (3, 1, 2, 0)  # deepest chunk first: warmest pipeline

            for b in range(B):
                nb = b + 1
                pieces = deque()
                if nb < B:
                    slabs[nb] = make_slabs(nb)
                    for i in range(NQC):
                        pieces.extend(qkv_pieces(nb, i, xt_load(nb, i)))
                if b == 3:
                    pieces.extend(outproj_pieces(0))
                    pieces.extend(outproj_pieces(1))
                for ci, qc_i in enumerate(QC_ORDER):
                    attention_qc(b, qc_i, filler=pieces)
                    # collective triggers are delayed one chunk past the last
                    # input write so the in-order gpsimd queue (which also
                    # carries cross-engine event semaphores) never blocks on
                    # them; batch-3's first half-exchange flies mid-batch
                    if ci == 0 and b == 2:
                        a2a(0)
                    elif ci == 0 and b == 3:
                        a2a(1)
                    elif ci == 1 and b == 3:
                        a2a(2)
                if b == 3:
                    a2a(3)
                while pieces:
                    pieces.popleft()()

            # batch-2's out-projection fills the final exchange's ~13us
            # rendezvous window (its own exchange completed a batch ago)
            for p_fn in outproj_pieces(2):
                p_fn()
            # batch-3 output projection: half A (chunks 3,1) is ready by now;
            # half B (chunks 2,0) overlaps the small final exchange
            for half in range(2):
                at3 = []
                for j in range(CORES):
                    a_t = apool.tile([P, P], BF16, tag=f"a3_{j}", name=f"a3_{half}_{j}")
                    nc.sync.dma_start(a_t[:], cc_out[2 + half][j])
                    at3.append(a_t)
                for nb_i in range(D // QC):
                    psum3 = ps_st.tile([P, 2, QC], F32, tag="st", name="o3psum")
                    psum = psum3[:, 0, :]
                    for j in range(CORES):
                        nc.tensor.matmul(
                            psum[:],
                            at3[j][:],
                            wo_t[:, j, nb_i * QC : (nb_i + 1) * QC],
                            start=(j == 0),
                            stop=False,
                        )
                    nc.tensor.matmul(
                        psum[:],
                        ones_t[0:1, 0:P],
                        bo_t[0:1, nb_i * QC : (nb_i + 1) * QC],
                        start=False,
                        stop=True,
                    )
                    o_s = epi.tile([P, QC], BF16, tag="o_s", name="o_s")
                    nc.vector.tensor_copy(o_s[:], psum[:])
                    nc.sync.dma_start(
                        out[
                            3 * SPC + half * P : 3 * SPC + (half + 1) * P,
                            nb_i * QC : (nb_i + 1) * QC,
                        ],
                        o_s[:],
                    )

    nc.compile()
    return nc


def _get_nc():
    if "nc" not in _CACHE:
        _CACHE["nc"] = _build()
    return _CACHE["nc"]


TRACE = False
LAST_RESULT = {}


def kernel(x, W_qkv, b_qkv, W_out, b_out):
    from concourse.bass_utils import run_bass_kernel_spmd
    import ml_dtypes

    x = np.asarray(x, dtype=np.float32)
    W_qkv = np.asarray(W_qkv, dtype=np.float32)
    b_qkv = np.asarray(b_qkv, dtype=np.float32)
    W_out = np.asarray(W_out, dtype=np.float32)
    b_out = np.asarray(b_out, dtype=np.float32)

    bf16 = ml_dtypes.bfloat16
    xt = np.ascontiguousarray(x.reshape(TOK, D).T).astype(bf16)  # [D, TOK]
    ident2 = np.ascontiguousarray(np.tile(np.eye(HD, dtype=np.float32), (2, 1)))
    wout_bf = W_out.astype(bf16)
    bout_bf = b_out.astype(bf16)
    # causal 0/1 masks for the 4 diagonal key-blocks of a 512-query chunk:
    # mask[jl][p, c] = 1 iff key (jl*128 + p) <= query c
    p_i = np.arange(P)[None, :, None]
    c_i = np.arange(QC)[None, None, :]
    jl_i = np.arange(DIAG)[:, None, None]
    maskd = (p_i + P * jl_i <= c_i).astype(np.float32).transpose(1, 0, 2)
    maskd = np.ascontiguousarray(maskd).astype(bf16)  # [P, DIAG, QC]

    in_maps = []
    for g in range(CORES):
        c = slice(g * P, (g + 1) * P)
        wq = np.concatenate(
            [W_qkv[:, c], W_qkv[:, D:][:, c], W_qkv[:, 2 * D:][:, c]], axis=1
        )
        bq = np.concatenate([b_qkv[c], b_qkv[D:][c], b_qkv[2 * D:][c]])
        in_maps.append(
            {
                "xt": xt,
                "wqkv": np.ascontiguousarray(wq).astype(bf16),
                "bqkv": np.ascontiguousarray(bq),
                "identf": ident2,
                "wout": wout_bf,
                "bout": bout_bf,
                "onesd": np.ones((P, P), dtype=bf16),
                "maskd": maskd,
            }
        )

    nc = _get_nc()
    res = run_bass_kernel_spmd(
        nc, in_maps, core_ids=list(range(CORES)), trace=TRACE
    )
    LAST_RESULT["res"] = res
    # core g owns the g-th 256-token slice of batches 0-2; for batch 3 it
    # owns the g-th 64-token piece of each chunk, ordered (3, 1, 2, 0)
    full = np.empty((B, S, D), dtype=np.float32)
    for g in range(CORES):
        og = np.asarray(res.results[g]["out"])  # [1024, D]
        ob = og[: 3 * SPC].reshape(3, SPC, D)
        for b in range(3):
            full[b, g * SPC : (g + 1) * SPC] = ob[b]
        o3 = og[3 * SPC :].reshape(4, HD, D)
        for idx, qc in enumerate((3, 1, 2, 0)):
            full[3, qc * QC + g * HD : qc * QC + (g + 1) * HD] = o3[idx]
    return full


# revision 82
# speedup vs baseline: 1.0113x; 1.0113x over previous
"""Multi-head self-attention (B=4, S=2048, D=1024, H=16, causal) on 8 TRN2 NeuronCores.

Sharding: tensor-parallel over heads (2 heads/core) for QKV projection + attention.
Per-batch AllToAll redistributes attention outputs so the output projection is
token-parallel (each core owns a 256-token slice of every batch). No reduction
collective needed.

Orientation: everything is computed transposed (feature-major) so all matmuls
contract over the partition dimension with 512-wide free dims:
  Q^T/K^T/V^T [hd, tok] = W^T x^T  (x^T supplied by host, bf16; V^T then
                                    PE-transposed per chunk to key-major
                                    V_aug tiles with a fused ones column)
  S^T [k, q]  = K^T-block as lhsT, Q^T as rhs (keys on partitions)
  P^T = exp(S^T/8) on ScalarE -> bf16, causal-masked by a DVE multiply with
        4 precomputed 0/1 diagonal-mask tiles (all-bf16 all-SBUF -> 2x mode)
  attn^T [hd, q] += V_aug^T P^T   (fused ones-column in V gives denominators)
  out [tok, d] = (attn^T chunks as lhsT) @ W_out

All matmul operands are bf16 (fp32 PSUM accumulation): the HAM power manager
clocks the PE at ~1.95 GHz for bf16 streams vs 1.2 GHz for fp32/fp32r.

Pipeline: batch b's attention interleaves batch b+1's QKV projection as PE
filler; a2a(b) fires right after batch b's attention; out-proj for batches
0-2 interleaves into batches 1 and 3 as more filler; only a2a(3)+outproj(3)
remain in the tail.
"""

import numpy as np

B, S, D, H = 4, 2048, 1024, 16
HD = D // H            # 64
CORES = 8
P = 128
TOK = B * S            # 8192 tokens (flattened b,s)
TPC = TOK // CORES     # 1024 tokens per core for out-proj
SPC = S // CORES       # 256-token slice of each batch owned per core
HPC = H // CORES       # 2 heads per core
QC = 512               # query chunk
NQC = S // QC          # 4 q-chunks per sequence
KB = S // P            # 16 key blocks per sequence
DCH = D // P           # 8 contraction chunks over D
DIAG = QC // P         # 4 diagonal key-blocks per q-chunk

_CACHE = {}


def _build():
    import concourse.mybir as mybir
    import concourse.tile as tile
    from concourse import bacc

    F32 = mybir.dt.float32
    BF16 = mybir.dt.bfloat16
    EXP = mybir.ActivationFunctionType.Exp
    MULT = mybir.AluOpType.mult

    nc = bacc.Bacc("TRN2", target_bir_lowering=False, debug=False, num_devices=CORES)

    xt = nc.dram_tensor("xt", [D, TOK], BF16, kind="ExternalInput").ap()
    wqkv = nc.dram_tensor("wqkv", [D, 3 * P], BF16, kind="ExternalInput").ap()
    bqkv = nc.dram_tensor("bqkv", [3 * P], F32, kind="ExternalInput").ap()
    identf = nc.dram_tensor("identf", [P, HD], F32, kind="ExternalInput").ap()
    wout = nc.dram_tensor("wout", [D, D], BF16, kind="ExternalInput").ap()
    bout = nc.dram_tensor("bout", [D], BF16, kind="ExternalInput").ap()
    onesd = nc.dram_tensor("onesd", [P, P], BF16, kind="ExternalInput").ap()
    maskd = nc.dram_tensor("maskd", [P, DIAG, QC], BF16, kind="ExternalInput").ap()
    out = nc.dram_tensor("out", [TPC, D], BF16, kind="ExternalOutput").ap()

    # A2A buffers: 4 exchanges (batches {0,1} / {2} / {3: chunks 3,1} /
    # {3: chunks 2,0}) so only the small final one is tail-exposed.
    # Slot j = this core's 2 heads x core j's token slices. For batch 3 the
    # ownership is 64-token interleaved so each exchange covers 2 chunks.
    cc_in = [
        nc.dram_tensor("cc_in01", [CORES, P, 2 * SPC], BF16),
        nc.dram_tensor("cc_in2", [CORES, P, SPC], BF16),
        nc.dram_tensor("cc_in3a", [CORES, P, P], BF16),
        nc.dram_tensor("cc_in3b", [CORES, P, P], BF16),
    ]
    cc_out = [
        nc.dram_tensor("cc_out01", [CORES, P, 2 * SPC], BF16),
        nc.dram_tensor("cc_out2", [CORES, P, SPC], BF16),
        nc.dram_tensor("cc_out3a", [CORES, P, P], BF16),
        nc.dram_tensor("cc_out3b", [CORES, P, P], BF16),
    ]

    with tile.TileContext(nc) as tc:
        with (
            tc.tile_pool(name="const", bufs=1) as const,
            tc.tile_pool(name="xpool", bufs=8) as xpool,
            tc.tile_pool(name="slab", bufs=2) as slab,
            tc.tile_pool(name="vpool", bufs=2) as vpool,
            tc.tile_pool(name="apool", bufs=2) as apool,
            tc.tile_pool(name="ppool", bufs=8) as ppool,
            tc.tile_pool(name="epi", bufs=3) as epi,
            tc.tile_pool(name="ps_st", bufs=3, space="PSUM") as ps_st,
            tc.tile_pool(name="ps_ot", bufs=2, space="PSUM") as ps_ot,
        ):
            # bias first (gates the opening bias-add), then W_qkv with the Q
            # columns leading; the masks/identity/ones aren't needed until
            # ~10us in, so they load after the startup-critical weights
            bq_t = const.tile([P, 3], F32)
            nc.sync.dma_start(bq_t[:], bqkv.rearrange("(s p) -> p s", p=P))
            wq_t = const.tile([P, DCH, 3 * P], BF16)
            wq_src = wqkv.rearrange("(o p) c -> p o c", p=P)
            nc.sync.dma_start(wq_t[:, :, 0:P], wq_src[:, :, 0:P])
            nc.sync.dma_start(wq_t[:, :, P:], wq_src[:, :, P:])
            ones_t = const.tile([P, P], BF16)
            nc.sync.dma_start(ones_t[:], onesd[:])
            idf_t = const.tile([P, HD], F32)
            nc.sync.dma_start(idf_t[:], identf[:])
            mask_t = const.tile([P, DIAG, QC], BF16)
            nc.sync.dma_start(mask_t[:], maskd[:])
            slabs = {}

            def make_slabs(b):
                vts = []
                for h in range(HPC):
                    vt = vpool.tile([P, KB, 66], BF16, tag=f"v2_{h}", name=f"v2_{h}_{b}")
                    nc.vector.tensor_copy(vt[:, :, 64:65], ones_t[:, 0:KB, None])
                    vts.append(vt)
                return (
                    slab.tile([P, S], BF16, tag="q2t", name=f"q2t{b}"),
                    slab.tile([P, S], BF16, tag="k2t", name=f"k2t{b}"),
                    slab.tile([P, S], F32, tag="v2t", name=f"v2t{b}"),
                    vts,
                )

            def xt_load(b, tc_i):
                """Prefetch the x^T chunk for (b, tc_i): issued a whole batch
                ahead so PE never waits on it even when a collective is
                hogging DMA bandwidth."""
                xt_t = xpool.tile([P, DCH, QC], BF16, tag="xt", name=f"xt_{b}_{tc_i}")
                t0 = b * S
                src = xt[:, t0 + tc_i * QC : t0 + (tc_i + 1) * QC].rearrange(
                    "(o p) t -> p o t", p=P
                )
                nc.sync.dma_start(xt_t[:, 0 : DCH // 2, :], src[:, 0 : DCH // 2, :])
                nc.sync.dma_start(xt_t[:, DCH // 2 :, :], src[:, DCH // 2 :, :])
                return xt_t

            def qkv_pieces(b, tc_i, xt_t):
                """QKV projection for token chunk tc_i of batch b, as 3 slab-level
                filler pieces operating on the prefetched x^T chunk."""
                q2t, k2t, v2t, vts = slabs[b]

                def piece(s_i, dst):
                    psum3 = ps_st.tile([P, 2, QC], F32, tag="st", name="qkvps")
                    psum = psum3[:, 0, :]
                    for dc in range(DCH):
                        nc.tensor.matmul(
                            psum[:],
                            wq_t[:, dc, s_i * P : (s_i + 1) * P],
                            xt_t[:, dc],
                            start=(dc == 0),
                            stop=(dc == DCH - 1),
                        )
                    nc.vector.tensor_scalar_add(
                        dst[:, tc_i * QC : (tc_i + 1) * QC],
                        psum[:],
                        bq_t[:, s_i : s_i + 1],
                    )
                    if s_i == 2:
                        # transpose this chunk's V^T block to key-major V_aug
                        # tiles (fp32 PE transpose; spread per-chunk so it
                        # rides the filler stream instead of batch bursts)
                        for h in range(HPC):
                            pst = ps_st.tile([P, 2, QC], F32, tag="st", name="vtp")
                            for j in range(DIAG):
                                kb = tc_i * DIAG + j
                                nc.tensor.transpose(
                                    pst[:, 0, j * HD : (j + 1) * HD],
                                    v2t[
                                        h * HD : (h + 1) * HD,
                                        kb * P : (kb + 1) * P,
                                    ],
                                    idf_t[h * HD : (h + 1) * HD, :],
                                )
                            nc.vector.tensor_copy(
                                vts[h][:, tc_i * DIAG : (tc_i + 1) * DIAG, 0:HD],
                                pst[:, 0, 0 : DIAG * HD].rearrange(
                                    "p (a b) -> p a b", b=HD
                                ),
                            )

                return [
                    (lambda s_i=s_i, dst=dst: piece(s_i, dst))
                    for s_i, dst in enumerate((q2t, k2t, v2t))
                ]

            def attention_qc(b, qc_i, filler=None):
                """Attention for q-chunk qc_i of batch b, both heads fused.

                filler: deque of independent-PE-work thunks; one is popped
                every 2nd group to plug exp-wait gaps (keeps the PE stream
                dense so the HAM clock stays up).
                """
                q2t, k2t, _, vts = slabs[b]
                qsl = slice(qc_i * QC, (qc_i + 1) * QC)
                nkb = (qc_i + 1) * DIAG
                otp = [
                    ps_ot.tile([P, QC], F32, tag="ot", name=f"ot{h}")
                    for h in range(HPC)
                ]
                for kb2 in range(nkb // 2):
                    tiles = []
                    for h in range(HPC):
                        stp = ps_st.tile([P, 2, QC], F32, tag="st", name=f"st{h}")
                        pt = ppool.tile([P, 2, QC], BF16, tag="pt", name=f"pt{h}")
                        tiles.append((stp, pt))
                    # scores: (headA, headB) pairs run concurrently (row groups 0/64)
                    for j in range(2):
                        kb = kb2 * 2 + j
                        for h in range(HPC):
                            hof = h * HD
                            nc.tensor.matmul(
                                tiles[h][0][:, j, :],
                                k2t[hof : hof + HD, kb * P : (kb + 1) * P],
                                q2t[hof : hof + HD, qsl],
                                start=True,
                                stop=True,
                                tile_position=(hof, 0),
                            )
                    for h in range(HPC):
                        stp, pt = tiles[h]
                        d0 = kb2 * 2 - qc_i * DIAG
                        if d0 + 1 >= 0:  # group touches the causal diagonal
                            # exp only the rectangle that can survive the
                            # mask; gpsimd (idle) zero-fills the fully-masked
                            # columns so the mask multiply sees no garbage.
                            # Shortens the chunk-end exp on the critical path.
                            for j in range(2):
                                z = P * (d0 + j)
                                if z > 0:
                                    nc.gpsimd.memset(pt[:, j, 0:z], 0.0)
                                nc.scalar.activation(
                                    pt[:, j, z:], stp[:, j, z:], EXP, scale=0.125
                                )
                            nc.vector.tensor_tensor(
                                pt[:], pt[:], mask_t[:, d0 : d0 + 2, :], MULT
                            )
                        else:
                            nc.scalar.activation(pt[:], stp[:], EXP, scale=0.125)
                    # bridge the exp->attnV latency with filler PE work; the
                    # in-order PE would otherwise stall on the last group of
                    # each chunk (nothing queued behind the diagonal's mask)
                    last = kb2 == nkb // 2 - 1
                    if filler and (last or kb2 % 2 == 0 or len(filler) > 8):
                        filler.popleft()()
                    for h in range(HPC):
                        pt = tiles[h][1]
                        for j in range(2):
                            kb = kb2 * 2 + j
                            nc.tensor.matmul(
                                otp[h][0:65, :],
                                vts[h][:, kb, 0:65],
                                pt[:, j, :],
                                start=(kb == 0),
                                stop=(kb == nkb - 1),
                            )
                for h in range(HPC):
                    hof = h * HD
                    # normalize by denominators (row 64): bcast via K=1 matmul
                    den_r = epi.tile([P, QC], BF16, tag="den_r", name="den_r")
                    nc.vector.tensor_copy(den_r[64:65, :], otp[h][64:65, :])
                    dbc3 = ps_st.tile([P, 2, QC], F32, tag="st", name="dbc3")
                    dbc = dbc3[:, 0, :]
                    nc.tensor.matmul(
                        dbc[0:HD, :], ones_t[64:65, 0:HD], den_r[64:65, :],
                        start=True, stop=True,
                    )
                    rden_s = epi.tile([HD, QC], F32, tag="rden_s", name="rden_s")
                    nc.vector.reciprocal_approx_fast(rden_s[:], dbc[0:HD, :])
                    attn_s = epi.tile([HD, QC], BF16, tag="attn_s", name="attn_s")
                    nc.vector.tensor_tensor(attn_s[:], otp[h][0:HD, :], rden_s[:], MULT)
                    # scatter token slices to the A2A input. Exchange 0
                    # carries batches 0+1 side by side; batch 3 is 64-token
                    # interleaved across two half-exchanges (3a: chunks 3,1;
                    # 3b: chunks 2,0) so 3a can fly mid-batch.
                    if b < 3:
                        cci = cc_in[0] if b < 2 else cc_in[1]
                        cof = (b % 2) * SPC if b < 2 else 0
                        for sl in range(2):
                            nc.sync.dma_start(
                                cci[
                                    2 * qc_i + sl,
                                    hof : hof + HD,
                                    cof : cof + SPC,
                                ],
                                attn_s[:, sl * SPC : (sl + 1) * SPC],
                            )
                    else:
                        cci = cc_in[2] if qc_i % 2 else cc_in[3]
                        cof = 0 if qc_i >= 2 else HD
                        for j in range(CORES):
                            nc.sync.dma_start(
                                cci[j, hof : hof + HD, cof : cof + HD],
                                attn_s[:, j * HD : (j + 1) * HD],
                            )

            def a2a(e):
                nc.gpsimd.collective_compute(
                    "AllToAll",
                    mybir.AluOpType.bypass,
                    replica_groups=[list(range(CORES))],
                    ins=[cc_in[e].ap().opt()],
                    outs=[cc_out[e].ap().opt()],
                )

            def outproj_pieces(b):
                """Output projection for this core's 256-token slice of batch b,
                as filler pieces. First piece loads the exchanged activations;
                the rest each compute one [128 tok, 512 feat] psum group."""
                cco = cc_out[0] if b < 2 else cc_out[b - 1]
                cof = (b % 2) * SPC if b < 2 else 0
                state = {}

                def load_piece():
                    at_b = []
                    for j in range(CORES):
                        a_t = apool.tile([P, SPC], BF16, tag=f"at{j}", name=f"at{b}_{j}")
                        nc.sync.dma_start(a_t[:], cco[j, :, cof : cof + SPC])
                        at_b.append(a_t)
                    state["at"] = at_b

                def group_piece(tb, nb_i):
                    at_b = state["at"]
                    psum3 = ps_st.tile([P, 2, QC], F32, tag="st", name="opsum")
                    psum = psum3[:, 0, :]
                    for j in range(CORES):
                        nc.tensor.matmul(
                            psum[:],
                            at_b[j][:, tb * P : (tb + 1) * P],
                            wo_t[:, j, nb_i * QC : (nb_i + 1) * QC],
                            start=(j == 0),
                            stop=False,
                        )
                    nc.tensor.matmul(
                        psum[:],
                        ones_t[0:1, 0:P],
                        bo_t[0:1, nb_i * QC : (nb_i + 1) * QC],
                        start=False,
                        stop=True,
                    )
                    o_s = epi.tile([P, QC], BF16, tag="o_s", name="o_s")
                    nc.vector.tensor_copy(o_s[:], psum[:])
                    nc.sync.dma_start(
                        out[
                            b * SPC + tb * P : b * SPC + (tb + 1) * P,
                            nb_i * QC : (nb_i + 1) * QC,
                        ],
                        o_s[:],
                    )

                return [load_piece] + [
                    (lambda tb=tb, nb_i=nb_i: group_piece(tb, nb_i))
                    for tb in range(SPC // P)
                    for nb_i in range(D // QC)
                ]

            # software pipeline: qkv(0) fully, then per batch interleave qkv(b+1)
            slabs[0] = make_slabs(0)
            xts0 = [xt_load(0, tc_i) for tc_i in range(NQC)]
            for tc_i in range(NQC):
                for p_fn in qkv_pieces(0, tc_i, xts0[tc_i]):
                    p_fn()
            # W_out loads off the startup critical path (needed ~batch 2)
            wo_t = const.tile([P, DCH, D], BF16)
            nc.sync.dma_start(wo_t[:], wout.rearrange("(o p) d -> p o d", p=P))
            bo_t = const.tile([1, D], BF16)
            nc.sync.dma_start(bo_t[:], bout[None, :])

            from collections import deque

            QC_ORDER = (3, 1, 2, 0)  # deepest chunk first: warmest pipeline

            for b in range(B):
                nb = b + 1
                pieces = deque()
                if nb < B:
                    slabs[nb] = make_slabs(nb)
                    for i in range(NQC):
                        pieces.extend(qkv_pieces(nb, i, xt_load(nb, i)))
                if b == 3:
                    pieces.extend(outproj_pieces(0))
                    pieces.extend(outproj_pieces(1))
                for ci, qc_i in enumerate(QC_ORDER):
                    attention_qc(b, qc_i, filler=pieces)
                    # collective triggers are delayed one chunk past the last
                    # input write so the in-order gpsimd queue (which also
                    # carries cross-engine event semaphores) never blocks on
                    # them; batch-3's first half-exchange flies mid-batch
                    if ci == 0 and b == 2:
                        a2a(0)
                    elif ci == 0 and b == 3:
                        a2a(1)
                    elif ci == 1 and b == 3:
                        a2a(2)
                if b == 3:
                    a2a(3)
                while pieces:
                    pieces.popleft()()

            # batch-2's out-projection fills the final exchange's ~13us
            # rendezvous window (its own exchange completed a batch ago)
            for p_fn in outproj_pieces(2):
                p_fn()
            # batch-3 output projection: half A (chunks 3,1) is ready by now;
            # half B (chunks 2,0) overlaps the small final exchange
            for half in range(2):
                at3 = []
                for j in range(CORES):
                    a_t = apool.tile([P, P], BF16, tag=f"a3_{j}", name=f"a3_{half}_{j}")
                    nc.sync.dma_start(a_t[:], cc_out[2 + half][j])
                    at3.append(a_t)
                for nb_i in range(D // QC):
                    psum3 = ps_st.tile([P, 2, QC], F32, tag="st", name="o3psum")
                    psum = psum3[:, 0, :]
                    for j in range(CORES):
                        nc.tensor.matmul(
                            psum[:],
                            at3[j][:],
                            wo_t[:, j, nb_i * QC : (nb_i + 1) * QC],
                            start=(j == 0),
                            stop=False,
                        )
                    nc.tensor.matmul(
                        psum[:],
                        ones_t[0:1, 0:P],
                        bo_t[0:1, nb_i * QC : (nb_i + 1) * QC],
                        start=False,
                        stop=True,
                    )
                    o_s = epi.tile([P, QC], BF16, tag="o_s", name="o_s")
                    nc.vector.tensor_copy(o_s[:], psum[:])
                    nc.sync.dma_start(
                        out[
                            3 * SPC + half * P : 3 * SPC + (half + 1) * P,
                            nb_i * QC : (nb_i + 1) * QC,
                        ],
                        o_s[:],
                    )

    nc.compile()
    return nc


def _get_nc():
    if "nc" not in _CACHE:
        _CACHE["nc"] = _build()
    return _CACHE["nc"]


TRACE = False
LAST_RESULT = {}


def kernel(x, W_qkv, b_qkv, W_out, b_out):
    from concourse.bass_utils import run_bass_kernel_spmd
    import ml_dtypes

    x = np.asarray(x, dtype=np.float32)
    W_qkv = np.asarray(W_qkv, dtype=np.float32)
    b_qkv = np.asarray(b_qkv, dtype=np.float32)
    W_out = np.asarray(W_out, dtype=np.float32)
    b_out = np.asarray(b_out, dtype=np.float32)

    bf16 = ml_dtypes.bfloat16
    xt = np.ascontiguousarray(x.reshape(TOK, D).T).astype(bf16)  # [D, TOK]
    ident2 = np.ascontiguousarray(np.tile(np.eye(HD, dtype=np.float32), (2, 1)))
    wout_bf = W_out.astype(bf16)
    bout_bf = b_out.astype(bf16)
    # causal 0/1 masks for the 4 diagonal key-blocks of a 512-query chunk:
    # mask[jl][p, c] = 1 iff key (jl*128 + p) <= query c
    p_i = np.arange(P)[None, :, None]
    c_i = np.arange(QC)[None, None, :]
    jl_i = np.arange(DIAG)[:, None, None]
    maskd = (p_i + P * jl_i <= c_i).astype(np.float32).transpose(1, 0, 2)
    maskd = np.ascontiguousarray(maskd).astype(bf16)  # [P, DIAG, QC]

    in_maps = []
    for g in range(CORES):
        c = slice(g * P, (g + 1) * P)
        wq = np.concatenate(
            [W_qkv[:, c], W_qkv[:, D:][:, c], W_qkv[:, 2 * D:][:, c]], axis=1
        )
        bq = np.concatenate([b_qkv[c], b_qkv[D:][c], b_qkv[2 * D:][c]])
        in_maps.append(
            {
                "xt": xt,
                "wqkv": np.ascontiguousarray(wq).astype(bf16),
                "bqkv": np.ascontiguousarray(bq),
                "identf": ident2,
                "wout": wout_bf,
                "bout": bout_bf,
                "onesd": np.ones((P, P), dtype=bf16),
                "maskd": maskd,
            }
        )

    nc = _get_nc()
    res = run_bass_kernel_spmd(
        nc, in_maps, core_ids=list(range(CORES)), trace=TRACE
    )
    LAST_RESULT["res"] = res
    # core g owns the g-th 256-token slice of batches 0-2; for batch 3 it
    # owns the g-th 64-token piece of each chunk, ordered (3, 1, 2, 0)
    full = np.empty((B, S, D), dtype=np.float32)
    for g in range(CORES):
        og = np.asarray(res.results[g]["out"])  # [1024, D]
        ob = og[: 3 * SPC].reshape(3, SPC, D)
        for b in range(3):
            full[b, g * SPC : (g + 1) * SPC] = ob[b]
        o3 = og[3 * SPC :].reshape(4, HD, D)
        for idx, qc in enumerate((3, 1, 2, 0)):
            full[3, qc * QC + g * HD : qc * QC + (g + 1) * HD] = o3[idx]
    return full
